# revision 39
# baseline (speedup 1.0000x reference)
"""Trainium2 bit-packing kernel (ConsolidateBits).

Input : x (4096, 32768) float32, uniform [0,1).
Output: (4096, 1024) uint32 — every 32 consecutive values along the last
dim packed into one word, bit i = (x > 0.5) at offset i.

Sharding: data-parallel over the batch dim, 512 rows per core, 8 cores.

Per-core schedule (cost model: engine queues run in parallel; DMA cost
charged to the issuing queue at ~332GB/s):
  in-DMAs  : each tile's 4MB split across SP / ACT (/ Pool early) queues,
             writing slices of one SBUF tile.
  compares : 8 per tile, one per bit position within a byte-octet, each
             pre-scaled by 2^m: c_m = (x[seg8 m] > 0.5) * 2^m -> bf16
             (TSP, 2x_2p on DVE).  Scales ride the compares for free, so
             the whole combine tree is pure adds.
  tree     : 7 TT adds per tile (pairs -> quads -> byte, every partial
             sum exact in bf16), split DVE/Pool for balance.
  tail     : half = byte_even + byte_odd*256 (TT mult by const tile +
             TT add -> i32, exact) on Pool; word = (half_odd << 16) |
             half_even as STT on DVE (gpsimd STT does not lower to HW).
  out-DMAs : adjacent output tiles pair-merged (one DMA per 2 tiles,
             strided DRAM AP), issued from SP/ACT.
First and last tiles are column-split with per-piece compute so the
pipeline ramps fast and the post-final-DMA serial chain is short.
"""

import sys

if "/opt/trn_rl_repo" not in sys.path:
    sys.path.insert(0, "/opt/trn_rl_repo")

import numpy as np

import concourse.bass as bass  # noqa: F401
import concourse.bacc as bacc
import concourse.mybir as mybir
from concourse.tile import TileContext
from concourse.alu_op_type import AluOpType as A
from concourse.bass_utils import run_bass_kernel_spmd

P = 128
N_CORES = 8
ROWS, COLS = 4096, 32768
ROWS_PER_CORE = ROWS // N_CORES   # 512
F = 8192                          # free-dim elements per partition per tile
NTILES = (ROWS_PER_CORE * COLS) // (P * F)  # 16
W = F // 32                       # words per partition per tile (256)

THR = 0.5


# per-tile sub-DMA chunk plans: Pool only carries input early (it is
# compute-bound later)
def tile_chunks(t):
    if 1 <= t <= 3:
        return ((0, 1792, "sp"), (1792, 1792, "sp"), (3584, 1792, "act"),
                (5376, 1792, "act"), (7168, 1024, "pool"))
    return ((0, 2048, "sp"), (2048, 2048, "sp"), (4096, 2048, "act"),
            (6144, 2048, "act"))


# ramp tile: small pieces, each with its own compute chain
RAMP_PIECES = ((0, 1024, "sp"), (1024, 1024, "act"), (2048, 1024, "sp"),
               (3072, 1024, "act"), (4096, 1024, "sp"), (5120, 1024, "act"),
               (6144, 1024, "pool"), (7168, 1024, "pool"))
# drain tile: descending pieces so every queue finishes together and the
# final serial chain is short
DRAIN_PIECES = ((0, 1024, "sp"), (1024, 1024, "act"), (2048, 1024, "sp"),
                (3072, 1024, "act"), (4096, 1024, "sp"), (5120, 1024, "act"),
                (6144, 1024, "sp"), (7168, 512, "act"), (7680, 256, "sp"),
                (7936, 256, "act"))
PIECE_PLANS = {0: RAMP_PIECES, 15: DRAIN_PIECES}
# tiles whose add-tree runs on DVE instead of Pool (balance)
# tiles whose compares run on Pool instead of DVE (balance)
CMP_POOL = (5, 11)
# tiles whose byte stage runs as bf16 TT-pair on Pool (else STT on DVE)
BYTE_POOL = ()
# output pair merging: (tiles, queue); singles for the last two tiles
OUT_PLAN = [((0, 1), "pool"), ((2, 3), "pool"), ((4, 5), "pool"),
            ((6, 7), "pool"), ((8, 9), "pool"), ((10, 11), "sp"),
            ((12, 13), "act"), ((14,), "sp"), ((15,), "act")]


def build(subs=None, piece_plans=None, cmp_pool=CMP_POOL,
          byte_pool=BYTE_POOL, out_plan=OUT_PLAN,
          x_bufs: int = 3, mid_bufs: int = 2, w_bufs: int = 3,
          p_bufs: int = 4):
    nc = bacc.Bacc("TRN2", target_bir_lowering=False)
    x = nc.dram_tensor(
        "x", [NTILES * P, F], mybir.dt.float32, kind="ExternalInput"
    )
    # int32 throughout the bitvec path (walrus: bitvec ops cannot cast);
    # reinterpreted as uint32 on the host.
    y = nc.dram_tensor(
        "y", [NTILES * P, W], mybir.dt.int32, kind="ExternalOutput"
    )
    xr = x[:, :].rearrange("(t p) f -> t p f", p=P)
    # pair view: partition p of pair q covers DRAM rows {q*256+p, q*256+128+p}
    ypair = y[:, :].rearrange("(q two p) w -> q p two w", two=2, p=P)
    ysingle = y[:, :].rearrange("(t p) w -> t p w", p=P)

    f32, bf16, i32 = mybir.dt.float32, mybir.dt.bfloat16, mybir.dt.int32

    with TileContext(nc) as tc:
        with (
            tc.tile_pool(name="consts", bufs=1) as cpool,
            tc.tile_pool(name="xpool", bufs=x_bufs) as xpool,
            tc.tile_pool(name="midpool", bufs=mid_bufs) as midpool,
            tc.tile_pool(name="piecepool", bufs=p_bufs) as piecepool,
            tc.tile_pool(name="wpool", bufs=w_bufs) as wpool,
        ):
            # Walrus requires bitvec-op scalars to be integer-typed and
            # match src/dst dtype; immediates lower as f32, so keep the
            # shift amount in a per-partition int32 const AP.
            shift16 = cpool.tile([P, 1], i32)
            nc.vector.memset(shift16[:], 16)
            c256 = cpool.tile([P, F // 16], bf16)
            nc.vector.memset(c256[:], 256.0)
            c16t = cpool.tile([P, F // 8], bf16)
            nc.vector.memset(c16t[:], 16.0)
            c16i = cpool.tile([P, F // 32], i32)
            nc.vector.memset(c16i[:], 16)

            dma_q = {"sp": nc.sync, "act": nc.scalar, "pool": nc.gpsimd}

            # word tiles, keyed by out_plan group; written sub-tile by
            # each member tile's final STT
            wt_tiles = {}
            for grp, _q in out_plan:
                wt_tiles[grp] = wpool.tile(
                    [P, W * len(grp)], i32, tag="wt", name=f"wt{grp[0]}"
                )

            def wt_slice(t, col0, fw):
                for grp, _q in out_plan:
                    if t in grp:
                        base = grp.index(t) * W
                        lo = base + col0 // 32
                        return wt_tiles[grp][:, lo : lo + fw // 32]
                raise KeyError(t)

            def compute(t, xt, col0, fw, mpool=None):
                """Full-tile chain: 4 pre-scaled compares (seg4 classes,
                scales 1,2,4,8; DVE TSP 2x_2p, or Pool for balance), a
                3-add tree on Pool (TT, hardware-proven on gpsimd), then
                byte/half/word as STTs on DVE (gpsimd STT does not lower
                to hardware)."""
                mpool = mpool or midpool
                seg = fw // 4
                xm = xt.rearrange("p (s m) -> p s m", m=4)
                cmp_eng = nc.gpsimd if t in cmp_pool else nc.vector

                # c_m = (x[seg4 m] > 0.5) * 2^m
                cls = mpool.tile([P, fw], bf16, tag="cls", name="cls")
                for m in range(4):
                    outm = cls[:, m * seg : (m + 1) * seg].rearrange(
                        "p (s one) -> p s one", one=1)
                    if m == 0:
                        cmp_eng.tensor_scalar(
                            out=outm, in0=xm[:, :, 0:1], scalar1=THR,
                            scalar2=None, op0=A.is_gt)
                    else:
                        cmp_eng.tensor_scalar(
                            out=outm, in0=xm[:, :, m : m + 1], scalar1=THR,
                            scalar2=float(1 << m), op0=A.is_gt, op1=A.mult)

                # tree on Pool: nib = (c0+c1) + (c2+c3), values <= 15
                t1 = mpool.tile([P, fw // 2], bf16, tag="t1", name="t1")
                for j in range(2):
                    nc.gpsimd.tensor_tensor(
                        out=t1[:, j * seg : (j + 1) * seg],
                        in0=cls[:, 2 * j * seg : (2 * j + 1) * seg],
                        in1=cls[:, (2 * j + 1) * seg : (2 * j + 2) * seg],
                        op=A.add)
                nib = mpool.tile([P, fw // 4], bf16, tag="nib", name="nib")
                nc.gpsimd.tensor_tensor(
                    out=nib[:], in0=t1[:, 0:seg], in1=t1[:, seg : 2 * seg],
                    op=A.add)

                # byte = nib_even + 16*nib_odd   (<=255, exact bf16)
                nv = nib[:].rearrange("p (k h) -> p k h", h=2)
                byt = mpool.tile([P, fw // 8], bf16, tag="byt", name="byt")
                if t in byte_pool:
                    # bf16-only TT pair on Pool (both ops HW-proven)
                    bs16 = mpool.tile([P, fw // 8], bf16, tag="bs16",
                                      name="bs16")
                    b16v = bs16[:].rearrange("p (k one) -> p k one", one=1)
                    nc.gpsimd.tensor_tensor(
                        out=b16v, in0=nv[:, :, 1:2],
                        in1=c16t[:, 0 : fw // 8].rearrange(
                            "p (k one) -> p k one", one=1),
                        op=A.mult)
                    nc.gpsimd.tensor_tensor(
                        out=byt[:].rearrange("p (k one) -> p k one", one=1),
                        in0=b16v, in1=nv[:, :, 0:1], op=A.add)
                else:
                    nc.vector.scalar_tensor_tensor(
                        out=byt[:].rearrange("p (k one) -> p k one", one=1),
                        in0=nv[:, :, 1:2], scalar=16.0, in1=nv[:, :, 0:1],
                        op0=A.mult, op1=A.add)

                # half = byte_even + 256*byte_odd  (exact, i32 out)
                bv = byt[:].rearrange("p (k h) -> p k h", h=2)
                half = mpool.tile([P, fw // 16], i32, tag="half", name="half")
                nc.vector.scalar_tensor_tensor(
                    out=half[:].rearrange("p (k one) -> p k one", one=1),
                    in0=bv[:, :, 1:2], scalar=256.0, in1=bv[:, :, 0:1],
                    op0=A.mult, op1=A.add)

                # word = (half_odd << 16) | half_even
                hs = half[:].rearrange("p (w h) -> p w h", h=2)
                nc.vector.scalar_tensor_tensor(
                    out=wt_slice(t, col0, fw).rearrange(
                        "p (w one) -> p w one", one=1),
                    in0=hs[:, :, 1:2], scalar=shift16[:],
                    in1=hs[:, :, 0:1],
                    op0=A.logical_shift_left, op1=A.bitwise_or,
                )

            def compute_piece(t, xt, col0, fw):
                """Compact 7-op all-DVE chain for ramp/drain pieces:
                short serial latency, STT allowed (DVE lowers fine)."""
                xv = xt.rearrange("p (s m) -> p s m", m=8)
                hi = piecepool.tile([P, fw // 2], bf16, tag="phi", name="phi")
                hiv = hi[:].rearrange("p (s m) -> p s m", m=4)
                nc.vector.tensor_scalar(
                    out=hiv, in0=xv[:, :, 4:8], scalar1=THR, scalar2=16.0,
                    op0=A.is_gt, op1=A.mult)
                o1 = piecepool.tile([P, fw // 2], bf16, tag="po1", name="po1")
                o1v = o1[:].rearrange("p (s m) -> p s m", m=4)
                nc.vector.tensor_scalar(
                    out=o1v, in0=xv[:, :, 0:4], scalar1=THR, scalar2=None,
                    op0=A.is_gt)
                nc.vector.tensor_tensor(
                    out=o1[:], in0=o1[:], in1=hi[:], op=A.add)
                o2 = piecepool.tile([P, fw // 4], bf16, tag="po2", name="po2")
                o1s = o1[:].rearrange("p (s m) -> p s m", m=4)
                nc.vector.scalar_tensor_tensor(
                    out=o2[:].rearrange("p (s m) -> p s m", m=2),
                    in0=o1s[:, :, 2:4], scalar=4.0, in1=o1s[:, :, 0:2],
                    op0=A.mult, op1=A.add)
                byt = piecepool.tile([P, fw // 8], bf16, tag="pby", name="pby")
                o2s = o2[:].rearrange("p (s m) -> p s m", m=2)
                nc.vector.scalar_tensor_tensor(
                    out=byt[:].rearrange("p (s one) -> p s one", one=1),
                    in0=o2s[:, :, 1:2], scalar=2.0, in1=o2s[:, :, 0:1],
                    op0=A.mult, op1=A.add)
                half = piecepool.tile([P, fw // 16], i32, tag="pha", name="pha")
                bys = byt[:].rearrange("p (k h) -> p k h", h=2)
                nc.vector.scalar_tensor_tensor(
                    out=half[:].rearrange("p (k one) -> p k one", one=1),
                    in0=bys[:, :, 1:2], scalar=256.0, in1=bys[:, :, 0:1],
                    op0=A.mult, op1=A.add)
                hs = half[:].rearrange("p (w h) -> p w h", h=2)
                nc.vector.scalar_tensor_tensor(
                    out=wt_slice(t, col0, fw).rearrange(
                        "p (w one) -> p w one", one=1),
                    in0=hs[:, :, 1:2], scalar=shift16[:],
                    in1=hs[:, :, 0:1],
                    op0=A.logical_shift_left, op1=A.bitwise_or,
                )

            def load(t, xt_ap, col0, fw, q):
                dma_q[q].dma_start(xt_ap, xr[t][:, col0 : col0 + fw])

            for t in range(NTILES):
                plans = PIECE_PLANS if piece_plans is None else piece_plans
                if t in plans:
                    # per-piece compute chains (ramp / drain)
                    for col0, fw, q in plans[t]:
                        xt = piecepool.tile(
                            [P, fw], f32, tag="xp", name="xp"
                        )
                        load(t, xt[:], col0, fw, q)
                        compute_piece(t, xt[:], col0, fw)
                else:
                    # one full-width tile filled by sub-DMA chunks
                    xt = xpool.tile([P, F], f32, tag="xt", name="xt")
                    for col0, fw, q in (subs or tile_chunks(t)):
                        load(t, xt[:, col0 : col0 + fw], col0, fw, q)
                    compute(t, xt[:], 0, F)

                # flush any output group this tile completes
                for grp, q in out_plan:
                    if grp[-1] == t:
                        wt = wt_tiles[grp]
                        if len(grp) == 2:
                            dma_q[q].dma_start(
                                ypair[grp[0] // 2],
                                wt[:].rearrange("p (two w) -> p two w", two=2),
                            )
                        else:
                            dma_q[q].dma_start(ysingle[grp[0]], wt[:])

    nc.compile()
    return nc


_NC_CACHE = {}


def _get_nc():
    if "nc" not in _NC_CACHE:
        _NC_CACHE["nc"] = build()
    return _NC_CACHE["nc"]


def _shard(x: np.ndarray):
    return [
        np.ascontiguousarray(
            x[i * ROWS_PER_CORE : (i + 1) * ROWS_PER_CORE].reshape(NTILES * P, F)
        )
        for i in range(N_CORES)
    ]


def run(x: np.ndarray, trace: bool = False):
    """Run the SPMD kernel; returns (full_output, BassKernelResults)."""
    nc = _get_nc()
    in_maps = [{"x": s} for s in _shard(x)]
    res = run_bass_kernel_spmd(nc, in_maps, core_ids=list(range(N_CORES)), trace=trace)
    parts = [
        np.asarray(m["y"]).view(np.uint32).reshape(ROWS_PER_CORE, COLS // 32)
        for m in res.results
    ]
    return np.concatenate(parts, axis=0), res


def kernel(x: np.ndarray) -> np.ndarray:
    out, _ = run(np.asarray(x, dtype=np.float32), trace=False)
    return out


# revision 40
# speedup vs baseline: 1.0017x; 1.0017x over previous
"""Trainium2 bit-packing kernel (ConsolidateBits).

Input : x (4096, 32768) float32, uniform [0,1).
Output: (4096, 1024) uint32 — every 32 consecutive values along the last
dim packed into one word, bit i = (x > 0.5) at offset i.

Sharding: data-parallel over the batch dim, 512 rows per core, 8 cores.

Per-core schedule (cost model: engine queues run in parallel; DMA cost
charged to the issuing queue at ~332GB/s):
  in-DMAs  : each tile's 4MB split across SP / ACT (/ Pool early) queues,
             writing slices of one SBUF tile.
  compares : 8 per tile, one per bit position within a byte-octet, each
             pre-scaled by 2^m: c_m = (x[seg8 m] > 0.5) * 2^m -> bf16
             (TSP, 2x_2p on DVE).  Scales ride the compares for free, so
             the whole combine tree is pure adds.
  tree     : 7 TT adds per tile (pairs -> quads -> byte, every partial
             sum exact in bf16), split DVE/Pool for balance.
  tail     : half = byte_even + byte_odd*256 (TT mult by const tile +
             TT add -> i32, exact) on Pool; word = (half_odd << 16) |
             half_even as STT on DVE (gpsimd STT does not lower to HW).
  out-DMAs : adjacent output tiles pair-merged (one DMA per 2 tiles,
             strided DRAM AP), issued from SP/ACT.
First and last tiles are column-split with per-piece compute so the
pipeline ramps fast and the post-final-DMA serial chain is short.
"""

import sys

if "/opt/trn_rl_repo" not in sys.path:
    sys.path.insert(0, "/opt/trn_rl_repo")

import numpy as np

import concourse.bass as bass  # noqa: F401
import concourse.bacc as bacc
import concourse.mybir as mybir
from concourse.tile import TileContext
from concourse.alu_op_type import AluOpType as A
from concourse.bass_utils import run_bass_kernel_spmd

P = 128
N_CORES = 8
ROWS, COLS = 4096, 32768
ROWS_PER_CORE = ROWS // N_CORES   # 512
F = 8192                          # free-dim elements per partition per tile
NTILES = (ROWS_PER_CORE * COLS) // (P * F)  # 16
W = F // 32                       # words per partition per tile (256)

THR = 0.5


# per-tile sub-DMA chunk plans: Pool only carries input early (it is
# compute-bound later)
def tile_chunks(t):
    if 1 <= t <= 3:
        return ((0, 1792, "sp"), (1792, 1792, "sp"), (3584, 1792, "act"),
                (5376, 1792, "act"), (7168, 1024, "pool"))
    return ((0, 2048, "sp"), (2048, 2048, "sp"), (4096, 2048, "act"),
            (6144, 2048, "act"))


# ramp tile: small pieces, each with its own compute chain
RAMP_PIECES = ((0, 1024, "sp"), (1024, 1024, "act"), (2048, 1024, "sp"),
               (3072, 1024, "act"), (4096, 1024, "sp"), (5120, 1024, "act"),
               (6144, 1024, "pool"), (7168, 1024, "pool"))
# drain tile: descending pieces so every queue finishes together and the
# final serial chain is short
DRAIN_PIECES = ((0, 1024, "sp"), (1024, 1024, "act"), (2048, 1024, "sp"),
                (3072, 1024, "act"), (4096, 1024, "sp"), (5120, 1024, "act"),
                (6144, 1024, "sp"), (7168, 512, "act"), (7680, 256, "sp"),
                (7936, 256, "act"))
PIECE_PLANS = {0: RAMP_PIECES, 15: DRAIN_PIECES}
# tiles whose add-tree runs on DVE instead of Pool (balance)
# tiles whose compares run on Pool instead of DVE (balance)
CMP_POOL = (5, 11)
# tiles whose byte stage runs as bf16 TT-pair on Pool (else STT on DVE)
BYTE_POOL = ()
# output pair merging: (tiles, queue); singles for the last two tiles
OUT_PLAN = [((0, 1), "pool"), ((2, 3), "pool"), ((4, 5), "pool"),
            ((6, 7), "pool"), ((8, 9), "pool"), ((10, 11), "sp"),
            ((12, 13), "sp"), ((14,), "act"), ((15,), "act")]


def build(subs=None, piece_plans=None, cmp_pool=CMP_POOL,
          byte_pool=BYTE_POOL, out_plan=OUT_PLAN,
          x_bufs: int = 3, mid_bufs: int = 2, w_bufs: int = 3,
          p_bufs: int = 4):
    nc = bacc.Bacc("TRN2", target_bir_lowering=False)
    x = nc.dram_tensor(
        "x", [NTILES * P, F], mybir.dt.float32, kind="ExternalInput"
    )
    # int32 throughout the bitvec path (walrus: bitvec ops cannot cast);
    # reinterpreted as uint32 on the host.
    y = nc.dram_tensor(
        "y", [NTILES * P, W], mybir.dt.int32, kind="ExternalOutput"
    )
    xr = x[:, :].rearrange("(t p) f -> t p f", p=P)
    # pair view: partition p of pair q covers DRAM rows {q*256+p, q*256+128+p}
    ypair = y[:, :].rearrange("(q two p) w -> q p two w", two=2, p=P)
    ysingle = y[:, :].rearrange("(t p) w -> t p w", p=P)

    f32, bf16, i32 = mybir.dt.float32, mybir.dt.bfloat16, mybir.dt.int32

    with TileContext(nc) as tc:
        with (
            tc.tile_pool(name="consts", bufs=1) as cpool,
            tc.tile_pool(name="xpool", bufs=x_bufs) as xpool,
            tc.tile_pool(name="midpool", bufs=mid_bufs) as midpool,
            tc.tile_pool(name="piecepool", bufs=p_bufs) as piecepool,
            tc.tile_pool(name="wpool", bufs=w_bufs) as wpool,
        ):
            # Walrus requires bitvec-op scalars to be integer-typed and
            # match src/dst dtype; immediates lower as f32, so keep the
            # shift amount in a per-partition int32 const AP.
            shift16 = cpool.tile([P, 1], i32)
            nc.vector.memset(shift16[:], 16)
            c256 = cpool.tile([P, F // 16], bf16)
            nc.vector.memset(c256[:], 256.0)
            c16t = cpool.tile([P, F // 8], bf16)
            nc.vector.memset(c16t[:], 16.0)
            c16i = cpool.tile([P, F // 32], i32)
            nc.vector.memset(c16i[:], 16)

            dma_q = {"sp": nc.sync, "act": nc.scalar, "pool": nc.gpsimd}

            # word tiles, keyed by out_plan group; written sub-tile by
            # each member tile's final STT
            wt_tiles = {}
            for grp, _q in out_plan:
                wt_tiles[grp] = wpool.tile(
                    [P, W * len(grp)], i32, tag="wt", name=f"wt{grp[0]}"
                )

            def wt_slice(t, col0, fw):
                for grp, _q in out_plan:
                    if t in grp:
                        base = grp.index(t) * W
                        lo = base + col0 // 32
                        return wt_tiles[grp][:, lo : lo + fw // 32]
                raise KeyError(t)

            def compute(t, xt, col0, fw, mpool=None):
                """Full-tile chain: 4 pre-scaled compares (seg4 classes,
                scales 1,2,4,8; DVE TSP 2x_2p, or Pool for balance), a
                3-add tree on Pool (TT, hardware-proven on gpsimd), then
                byte/half/word as STTs on DVE (gpsimd STT does not lower
                to hardware)."""
                mpool = mpool or midpool
                seg = fw // 4
                xm = xt.rearrange("p (s m) -> p s m", m=4)
                cmp_eng = nc.gpsimd if t in cmp_pool else nc.vector

                # c_m = (x[seg4 m] > 0.5) * 2^m
                cls = mpool.tile([P, fw], bf16, tag="cls", name="cls")
                for m in range(4):
                    outm = cls[:, m * seg : (m + 1) * seg].rearrange(
                        "p (s one) -> p s one", one=1)
                    if m == 0:
                        cmp_eng.tensor_scalar(
                            out=outm, in0=xm[:, :, 0:1], scalar1=THR,
                            scalar2=None, op0=A.is_gt)
                    else:
                        cmp_eng.tensor_scalar(
                            out=outm, in0=xm[:, :, m : m + 1], scalar1=THR,
                            scalar2=float(1 << m), op0=A.is_gt, op1=A.mult)

                # tree on Pool: nib = (c0+c1) + (c2+c3), values <= 15
                t1 = mpool.tile([P, fw // 2], bf16, tag="t1", name="t1")
                for j in range(2):
                    nc.gpsimd.tensor_tensor(
                        out=t1[:, j * seg : (j + 1) * seg],
                        in0=cls[:, 2 * j * seg : (2 * j + 1) * seg],
                        in1=cls[:, (2 * j + 1) * seg : (2 * j + 2) * seg],
                        op=A.add)
                nib = mpool.tile([P, fw // 4], bf16, tag="nib", name="nib")
                nc.gpsimd.tensor_tensor(
                    out=nib[:], in0=t1[:, 0:seg], in1=t1[:, seg : 2 * seg],
                    op=A.add)

                # byte = nib_even + 16*nib_odd   (<=255, exact bf16)
                nv = nib[:].rearrange("p (k h) -> p k h", h=2)
                byt = mpool.tile([P, fw // 8], bf16, tag="byt", name="byt")
                if t in byte_pool:
                    # bf16-only TT pair on Pool (both ops HW-proven)
                    bs16 = mpool.tile([P, fw // 8], bf16, tag="bs16",
                                      name="bs16")
                    b16v = bs16[:].rearrange("p (k one) -> p k one", one=1)
                    nc.gpsimd.tensor_tensor(
                        out=b16v, in0=nv[:, :, 1:2],
                        in1=c16t[:, 0 : fw // 8].rearrange(
                            "p (k one) -> p k one", one=1),
                        op=A.mult)
                    nc.gpsimd.tensor_tensor(
                        out=byt[:].rearrange("p (k one) -> p k one", one=1),
                        in0=b16v, in1=nv[:, :, 0:1], op=A.add)
                else:
                    nc.vector.scalar_tensor_tensor(
                        out=byt[:].rearrange("p (k one) -> p k one", one=1),
                        in0=nv[:, :, 1:2], scalar=16.0, in1=nv[:, :, 0:1],
                        op0=A.mult, op1=A.add)

                # half = byte_even + 256*byte_odd  (exact, i32 out)
                bv = byt[:].rearrange("p (k h) -> p k h", h=2)
                half = mpool.tile([P, fw // 16], i32, tag="half", name="half")
                nc.vector.scalar_tensor_tensor(
                    out=half[:].rearrange("p (k one) -> p k one", one=1),
                    in0=bv[:, :, 1:2], scalar=256.0, in1=bv[:, :, 0:1],
                    op0=A.mult, op1=A.add)

                # word = (half_odd << 16) | half_even
                hs = half[:].rearrange("p (w h) -> p w h", h=2)
                nc.vector.scalar_tensor_tensor(
                    out=wt_slice(t, col0, fw).rearrange(
                        "p (w one) -> p w one", one=1),
                    in0=hs[:, :, 1:2], scalar=shift16[:],
                    in1=hs[:, :, 0:1],
                    op0=A.logical_shift_left, op1=A.bitwise_or,
                )

            def compute_piece(t, xt, col0, fw):
                """Compact 7-op all-DVE chain for ramp/drain pieces:
                short serial latency, STT allowed (DVE lowers fine)."""
                xv = xt.rearrange("p (s m) -> p s m", m=8)
                hi = piecepool.tile([P, fw // 2], bf16, tag="phi", name="phi")
                hiv = hi[:].rearrange("p (s m) -> p s m", m=4)
                nc.vector.tensor_scalar(
                    out=hiv, in0=xv[:, :, 4:8], scalar1=THR, scalar2=16.0,
                    op0=A.is_gt, op1=A.mult)
                o1 = piecepool.tile([P, fw // 2], bf16, tag="po1", name="po1")
                o1v = o1[:].rearrange("p (s m) -> p s m", m=4)
                nc.vector.tensor_scalar(
                    out=o1v, in0=xv[:, :, 0:4], scalar1=THR, scalar2=None,
                    op0=A.is_gt)
                nc.vector.tensor_tensor(
                    out=o1[:], in0=o1[:], in1=hi[:], op=A.add)
                o2 = piecepool.tile([P, fw // 4], bf16, tag="po2", name="po2")
                o1s = o1[:].rearrange("p (s m) -> p s m", m=4)
                nc.vector.scalar_tensor_tensor(
                    out=o2[:].rearrange("p (s m) -> p s m", m=2),
                    in0=o1s[:, :, 2:4], scalar=4.0, in1=o1s[:, :, 0:2],
                    op0=A.mult, op1=A.add)
                byt = piecepool.tile([P, fw // 8], bf16, tag="pby", name="pby")
                o2s = o2[:].rearrange("p (s m) -> p s m", m=2)
                nc.vector.scalar_tensor_tensor(
                    out=byt[:].rearrange("p (s one) -> p s one", one=1),
                    in0=o2s[:, :, 1:2], scalar=2.0, in1=o2s[:, :, 0:1],
                    op0=A.mult, op1=A.add)
                half = piecepool.tile([P, fw // 16], i32, tag="pha", name="pha")
                bys = byt[:].rearrange("p (k h) -> p k h", h=2)
                nc.vector.scalar_tensor_tensor(
                    out=half[:].rearrange("p (k one) -> p k one", one=1),
                    in0=bys[:, :, 1:2], scalar=256.0, in1=bys[:, :, 0:1],
                    op0=A.mult, op1=A.add)
                hs = half[:].rearrange("p (w h) -> p w h", h=2)
                nc.vector.scalar_tensor_tensor(
                    out=wt_slice(t, col0, fw).rearrange(
                        "p (w one) -> p w one", one=1),
                    in0=hs[:, :, 1:2], scalar=shift16[:],
                    in1=hs[:, :, 0:1],
                    op0=A.logical_shift_left, op1=A.bitwise_or,
                )

            def load(t, xt_ap, col0, fw, q):
                dma_q[q].dma_start(xt_ap, xr[t][:, col0 : col0 + fw])

            for t in range(NTILES):
                plans = PIECE_PLANS if piece_plans is None else piece_plans
                if t in plans:
                    # per-piece compute chains (ramp / drain)
                    for col0, fw, q in plans[t]:
                        xt = piecepool.tile(
                            [P, fw], f32, tag="xp", name="xp"
                        )
                        load(t, xt[:], col0, fw, q)
                        compute_piece(t, xt[:], col0, fw)
                else:
                    # one full-width tile filled by sub-DMA chunks
                    xt = xpool.tile([P, F], f32, tag="xt", name="xt")
                    for col0, fw, q in (subs or tile_chunks(t)):
                        load(t, xt[:, col0 : col0 + fw], col0, fw, q)
                    compute(t, xt[:], 0, F)

                # flush any output group this tile completes
                for grp, q in out_plan:
                    if grp[-1] == t:
                        wt = wt_tiles[grp]
                        if len(grp) == 2:
                            dma_q[q].dma_start(
                                ypair[grp[0] // 2],
                                wt[:].rearrange("p (two w) -> p two w", two=2),
                            )
                        else:
                            dma_q[q].dma_start(ysingle[grp[0]], wt[:])

    nc.compile()
    return nc


_NC_CACHE = {}


def _get_nc():
    if "nc" not in _NC_CACHE:
        _NC_CACHE["nc"] = build()
    return _NC_CACHE["nc"]


def _shard(x: np.ndarray):
    return [
        np.ascontiguousarray(
            x[i * ROWS_PER_CORE : (i + 1) * ROWS_PER_CORE].reshape(NTILES * P, F)
        )
        for i in range(N_CORES)
    ]


def run(x: np.ndarray, trace: bool = False):
    """Run the SPMD kernel; returns (full_output, BassKernelResults)."""
    nc = _get_nc()
    in_maps = [{"x": s} for s in _shard(x)]
    res = run_bass_kernel_spmd(nc, in_maps, core_ids=list(range(N_CORES)), trace=trace)
    parts = [
        np.asarray(m["y"]).view(np.uint32).reshape(ROWS_PER_CORE, COLS // 32)
        for m in res.results
    ]
    return np.concatenate(parts, axis=0), res


def kernel(x: np.ndarray) -> np.ndarray:
    out, _ = run(np.asarray(x, dtype=np.float32), trace=False)
    return out


# revision 42
# speedup vs baseline: 1.0017x; 1.0000x over previous
"""Trainium2 bit-packing kernel (ConsolidateBits).

Input : x (4096, 32768) float32, uniform [0,1).
Output: (4096, 1024) uint32 — every 32 consecutive values along the last
dim packed into one word, bit i = (x > 0.5) at offset i.

Sharding: data-parallel over the batch dim, 512 rows per core, 8 cores.

Per-core schedule (cost model: engine queues run in parallel; DMA cost
charged to the issuing queue at ~332GB/s):
  in-DMAs  : each tile's 4MB split across SP / ACT (/ Pool early) queues,
             writing slices of one SBUF tile.
  compares : 8 per tile, one per bit position within a byte-octet, each
             pre-scaled by 2^m: c_m = (x[seg8 m] > 0.5) * 2^m -> bf16
             (TSP, 2x_2p on DVE).  Scales ride the compares for free, so
             the whole combine tree is pure adds.
  tree     : 7 TT adds per tile (pairs -> quads -> byte, every partial
             sum exact in bf16), split DVE/Pool for balance.
  tail     : half = byte_even + byte_odd*256 (TT mult by const tile +
             TT add -> i32, exact) on Pool; word = (half_odd << 16) |
             half_even as STT on DVE (gpsimd STT does not lower to HW).
  out-DMAs : adjacent output tiles pair-merged (one DMA per 2 tiles,
             strided DRAM AP), issued from SP/ACT.
First and last tiles are column-split with per-piece compute so the
pipeline ramps fast and the post-final-DMA serial chain is short.
"""

import sys

if "/opt/trn_rl_repo" not in sys.path:
    sys.path.insert(0, "/opt/trn_rl_repo")

import numpy as np

import concourse.bass as bass  # noqa: F401
import concourse.bacc as bacc
import concourse.mybir as mybir
from concourse.tile import TileContext
from concourse.alu_op_type import AluOpType as A
from concourse.bass_utils import run_bass_kernel_spmd

P = 128
N_CORES = 8
ROWS, COLS = 4096, 32768
ROWS_PER_CORE = ROWS // N_CORES   # 512
F = 8192                          # free-dim elements per partition per tile
NTILES = (ROWS_PER_CORE * COLS) // (P * F)  # 16
W = F // 32                       # words per partition per tile (256)

THR = 0.5


# per-tile sub-DMA chunk plans: Pool only carries input early (it is
# compute-bound later)
def tile_chunks(t):
    if 1 <= t <= 3:
        return ((0, 1792, "sp"), (1792, 1792, "sp"), (3584, 1792, "act"),
                (5376, 1792, "act"), (7168, 1024, "pool"))
    return ((0, 2048, "sp"), (2048, 2048, "sp"), (4096, 2048, "act"),
            (6144, 2048, "act"))


# ramp tile: small pieces, each with its own compute chain
RAMP_PIECES = ((0, 1024, "sp"), (1024, 1024, "act"), (2048, 1024, "sp"),
               (3072, 1024, "act"), (4096, 1024, "sp"), (5120, 1024, "act"),
               (6144, 1024, "pool"), (7168, 1024, "pool"))
# drain tile: descending pieces so every queue finishes together and the
# final serial chain is short
DRAIN_PIECES = ((0, 1024, "sp"), (1024, 1024, "act"), (2048, 1024, "sp"),
                (3072, 1024, "act"), (4096, 1024, "sp"), (5120, 1024, "act"),
                (6144, 512, "sp"), (6656, 512, "act"), (7168, 512, "sp"),
                (7680, 512, "act"))
PIECE_PLANS = {0: RAMP_PIECES, 15: DRAIN_PIECES}
# tiles whose add-tree runs on DVE instead of Pool (balance)
# tiles whose compares run on Pool instead of DVE (balance)
CMP_POOL = (5, 11)
# tiles whose byte stage runs as bf16 TT-pair on Pool (else STT on DVE)
BYTE_POOL = ()
# output pair merging: (tiles, queue); singles for the last two tiles
OUT_PLAN = [((0, 1), "pool"), ((2, 3), "pool"), ((4, 5), "pool"),
            ((6, 7), "pool"), ((8, 9), "pool"), ((10, 11), "sp"),
            ((12, 13), "sp"), ((14,), "act"), ((15,), "act")]


def build(subs=None, piece_plans=None, cmp_pool=CMP_POOL,
          byte_pool=BYTE_POOL, out_plan=OUT_PLAN,
          x_bufs: int = 3, mid_bufs: int = 2, w_bufs: int = 3,
          p_bufs: int = 4, cls_bufs: int = 2):
    nc = bacc.Bacc("TRN2", target_bir_lowering=False)
    x = nc.dram_tensor(
        "x", [NTILES * P, F], mybir.dt.float32, kind="ExternalInput"
    )
    # int32 throughout the bitvec path (walrus: bitvec ops cannot cast);
    # reinterpreted as uint32 on the host.
    y = nc.dram_tensor(
        "y", [NTILES * P, W], mybir.dt.int32, kind="ExternalOutput"
    )
    xr = x[:, :].rearrange("(t p) f -> t p f", p=P)
    # pair view: partition p of pair q covers DRAM rows {q*256+p, q*256+128+p}
    ypair = y[:, :].rearrange("(q two p) w -> q p two w", two=2, p=P)
    ysingle = y[:, :].rearrange("(t p) w -> t p w", p=P)

    f32, bf16, i32 = mybir.dt.float32, mybir.dt.bfloat16, mybir.dt.int32

    with TileContext(nc) as tc:
        with (
            tc.tile_pool(name="consts", bufs=1) as cpool,
            tc.tile_pool(name="xpool", bufs=x_bufs) as xpool,
            tc.tile_pool(name="midpool", bufs=mid_bufs) as midpool,
            tc.tile_pool(name="piecepool", bufs=p_bufs) as piecepool,
            tc.tile_pool(name="wpool", bufs=w_bufs) as wpool,
        ):
            # Walrus requires bitvec-op scalars to be integer-typed and
            # match src/dst dtype; immediates lower as f32, so keep the
            # shift amount in a per-partition int32 const AP.
            shift16 = cpool.tile([P, 1], i32)
            nc.vector.memset(shift16[:], 16)
            c256 = cpool.tile([P, F // 16], bf16)
            nc.vector.memset(c256[:], 256.0)
            c16t = cpool.tile([P, F // 8], bf16)
            nc.vector.memset(c16t[:], 16.0)
            c16i = cpool.tile([P, F // 32], i32)
            nc.vector.memset(c16i[:], 16)

            dma_q = {"sp": nc.sync, "act": nc.scalar, "pool": nc.gpsimd}

            # word tiles, keyed by out_plan group; written sub-tile by
            # each member tile's final STT
            wt_tiles = {}
            for grp, _q in out_plan:
                wt_tiles[grp] = wpool.tile(
                    [P, W * len(grp)], i32, tag="wt", name=f"wt{grp[0]}"
                )

            def wt_slice(t, col0, fw):
                for grp, _q in out_plan:
                    if t in grp:
                        base = grp.index(t) * W
                        lo = base + col0 // 32
                        return wt_tiles[grp][:, lo : lo + fw // 32]
                raise KeyError(t)

            def compute(t, xt, col0, fw, mpool=None):
                """Full-tile chain: 4 pre-scaled compares (seg4 classes,
                scales 1,2,4,8; DVE TSP 2x_2p, or Pool for balance), a
                3-add tree on Pool (TT, hardware-proven on gpsimd), then
                byte/half/word as STTs on DVE (gpsimd STT does not lower
                to hardware)."""
                mpool = mpool or midpool
                seg = fw // 4
                xm = xt.rearrange("p (s m) -> p s m", m=4)
                cmp_eng = nc.gpsimd if t in cmp_pool else nc.vector

                # c_m = (x[seg4 m] > 0.5) * 2^m
                cls = mpool.tile([P, fw], bf16, tag="cls", name="cls",
                                 bufs=cls_bufs)
                for m in range(4):
                    outm = cls[:, m * seg : (m + 1) * seg].rearrange(
                        "p (s one) -> p s one", one=1)
                    if m == 0:
                        cmp_eng.tensor_scalar(
                            out=outm, in0=xm[:, :, 0:1], scalar1=THR,
                            scalar2=None, op0=A.is_gt)
                    else:
                        cmp_eng.tensor_scalar(
                            out=outm, in0=xm[:, :, m : m + 1], scalar1=THR,
                            scalar2=float(1 << m), op0=A.is_gt, op1=A.mult)

                # tree on Pool: nib = (c0+c1) + (c2+c3), values <= 15
                t1 = mpool.tile([P, fw // 2], bf16, tag="t1", name="t1")
                for j in range(2):
                    nc.gpsimd.tensor_tensor(
                        out=t1[:, j * seg : (j + 1) * seg],
                        in0=cls[:, 2 * j * seg : (2 * j + 1) * seg],
                        in1=cls[:, (2 * j + 1) * seg : (2 * j + 2) * seg],
                        op=A.add)
                nib = mpool.tile([P, fw // 4], bf16, tag="nib", name="nib")
                nc.gpsimd.tensor_tensor(
                    out=nib[:], in0=t1[:, 0:seg], in1=t1[:, seg : 2 * seg],
                    op=A.add)

                # byte = nib_even + 16*nib_odd   (<=255, exact bf16)
                nv = nib[:].rearrange("p (k h) -> p k h", h=2)
                byt = mpool.tile([P, fw // 8], bf16, tag="byt", name="byt")
                if t in byte_pool:
                    # bf16-only TT pair on Pool (both ops HW-proven)
                    bs16 = mpool.tile([P, fw // 8], bf16, tag="bs16",
                                      name="bs16")
                    b16v = bs16[:].rearrange("p (k one) -> p k one", one=1)
                    nc.gpsimd.tensor_tensor(
                        out=b16v, in0=nv[:, :, 1:2],
                        in1=c16t[:, 0 : fw // 8].rearrange(
                            "p (k one) -> p k one", one=1),
                        op=A.mult)
                    nc.gpsimd.tensor_tensor(
                        out=byt[:].rearrange("p (k one) -> p k one", one=1),
                        in0=b16v, in1=nv[:, :, 0:1], op=A.add)
                else:
                    nc.vector.scalar_tensor_tensor(
                        out=byt[:].rearrange("p (k one) -> p k one", one=1),
                        in0=nv[:, :, 1:2], scalar=16.0, in1=nv[:, :, 0:1],
                        op0=A.mult, op1=A.add)

                # half = byte_even + 256*byte_odd  (exact, i32 out)
                bv = byt[:].rearrange("p (k h) -> p k h", h=2)
                half = mpool.tile([P, fw // 16], i32, tag="half", name="half")
                nc.vector.scalar_tensor_tensor(
                    out=half[:].rearrange("p (k one) -> p k one", one=1),
                    in0=bv[:, :, 1:2], scalar=256.0, in1=bv[:, :, 0:1],
                    op0=A.mult, op1=A.add)

                # word = (half_odd << 16) | half_even
                hs = half[:].rearrange("p (w h) -> p w h", h=2)
                nc.vector.scalar_tensor_tensor(
                    out=wt_slice(t, col0, fw).rearrange(
                        "p (w one) -> p w one", one=1),
                    in0=hs[:, :, 1:2], scalar=shift16[:],
                    in1=hs[:, :, 0:1],
                    op0=A.logical_shift_left, op1=A.bitwise_or,
                )

            def compute_piece(t, xt, col0, fw):
                """Compact 7-op all-DVE chain for ramp/drain pieces:
                short serial latency, STT allowed (DVE lowers fine)."""
                xv = xt.rearrange("p (s m) -> p s m", m=8)
                hi = piecepool.tile([P, fw // 2], bf16, tag="phi", name="phi")
                hiv = hi[:].rearrange("p (s m) -> p s m", m=4)
                nc.vector.tensor_scalar(
                    out=hiv, in0=xv[:, :, 4:8], scalar1=THR, scalar2=16.0,
                    op0=A.is_gt, op1=A.mult)
                o1 = piecepool.tile([P, fw // 2], bf16, tag="po1", name="po1")
                o1v = o1[:].rearrange("p (s m) -> p s m", m=4)
                nc.vector.tensor_scalar(
                    out=o1v, in0=xv[:, :, 0:4], scalar1=THR, scalar2=None,
                    op0=A.is_gt)
                nc.vector.tensor_tensor(
                    out=o1[:], in0=o1[:], in1=hi[:], op=A.add)
                o2 = piecepool.tile([P, fw // 4], bf16, tag="po2", name="po2")
                o1s = o1[:].rearrange("p (s m) -> p s m", m=4)
                nc.vector.scalar_tensor_tensor(
                    out=o2[:].rearrange("p (s m) -> p s m", m=2),
                    in0=o1s[:, :, 2:4], scalar=4.0, in1=o1s[:, :, 0:2],
                    op0=A.mult, op1=A.add)
                byt = piecepool.tile([P, fw // 8], bf16, tag="pby", name="pby")
                o2s = o2[:].rearrange("p (s m) -> p s m", m=2)
                nc.vector.scalar_tensor_tensor(
                    out=byt[:].rearrange("p (s one) -> p s one", one=1),
                    in0=o2s[:, :, 1:2], scalar=2.0, in1=o2s[:, :, 0:1],
                    op0=A.mult, op1=A.add)
                half = piecepool.tile([P, fw // 16], i32, tag="pha", name="pha")
                bys = byt[:].rearrange("p (k h) -> p k h", h=2)
                nc.vector.scalar_tensor_tensor(
                    out=half[:].rearrange("p (k one) -> p k one", one=1),
                    in0=bys[:, :, 1:2], scalar=256.0, in1=bys[:, :, 0:1],
                    op0=A.mult, op1=A.add)
                hs = half[:].rearrange("p (w h) -> p w h", h=2)
                nc.vector.scalar_tensor_tensor(
                    out=wt_slice(t, col0, fw).rearrange(
                        "p (w one) -> p w one", one=1),
                    in0=hs[:, :, 1:2], scalar=shift16[:],
                    in1=hs[:, :, 0:1],
                    op0=A.logical_shift_left, op1=A.bitwise_or,
                )

            def load(t, xt_ap, col0, fw, q):
                dma_q[q].dma_start(xt_ap, xr[t][:, col0 : col0 + fw])

            for t in range(NTILES):
                plans = PIECE_PLANS if piece_plans is None else piece_plans
                if t in plans:
                    # per-piece compute chains (ramp / drain)
                    for col0, fw, q in plans[t]:
                        xt = piecepool.tile(
                            [P, fw], f32, tag="xp", name="xp"
                        )
                        load(t, xt[:], col0, fw, q)
                        compute_piece(t, xt[:], col0, fw)
                else:
                    # one full-width tile filled by sub-DMA chunks
                    xt = xpool.tile([P, F], f32, tag="xt", name="xt")
                    for col0, fw, q in (subs or tile_chunks(t)):
                        load(t, xt[:, col0 : col0 + fw], col0, fw, q)
                    compute(t, xt[:], 0, F)

                # flush any output group this tile completes
                for grp, q in out_plan:
                    if grp[-1] == t:
                        wt = wt_tiles[grp]
                        if len(grp) == 2:
                            dma_q[q].dma_start(
                                ypair[grp[0] // 2],
                                wt[:].rearrange("p (two w) -> p two w", two=2),
                            )
                        else:
                            dma_q[q].dma_start(ysingle[grp[0]], wt[:])

    nc.compile()
    return nc


_NC_CACHE = {}


def _get_nc():
    if "nc" not in _NC_CACHE:
        _NC_CACHE["nc"] = build()
    return _NC_CACHE["nc"]


def _shard(x: np.ndarray):
    return [
        np.ascontiguousarray(
            x[i * ROWS_PER_CORE : (i + 1) * ROWS_PER_CORE].reshape(NTILES * P, F)
        )
        for i in range(N_CORES)
    ]


def run(x: np.ndarray, trace: bool = False):
    """Run the SPMD kernel; returns (full_output, BassKernelResults)."""
    nc = _get_nc()
    in_maps = [{"x": s} for s in _shard(x)]
    res = run_bass_kernel_spmd(nc, in_maps, core_ids=list(range(N_CORES)), trace=trace)
    parts = [
        np.asarray(m["y"]).view(np.uint32).reshape(ROWS_PER_CORE, COLS // 32)
        for m in res.results
    ]
    return np.concatenate(parts, axis=0), res


def kernel(x: np.ndarray) -> np.ndarray:
    out, _ = run(np.asarray(x, dtype=np.float32), trace=False)
    return out
